# revision 35
# baseline (speedup 1.0000x reference)
"""Trainium2 Bass kernel for nn_DecoupledPointJAFAR.

Self-contained: takes FULL inputs, shards across 8 NeuronCores internally
(core c -> batch c//4, hr-point slice (c%4)*4096), returns FULL outputs
(out (B,64,N) f32, bdy_prob (B,1,N) f32).

Per core (B=2, N=16384, M=4096, k=16, C=64):
  1. kNN scores s = 2 x.y - |y|^2 via exact triple-bf16 decomposition
     matmul (K=30, products exact, f32 PSUM accumulate) -> top-16 per
     point via DVE max8/max_index/match_replace (two rounds) on PSUM.
  2. Conv chains as bf16 matmuls; biases/BN folded into ACT
     activation(scale, bias) per-partition args.
  3. Gathered attention: [K|C''] table (bf16 256B rows) gathered
     channel-major via SBUF-source dma_gather(transpose=True);
     logits = Q.K_g + Qt.relu(A''_n + C''_m) + Q.b2, with channel
     reductions as ones-matmuls accumulating into shared PSUM tiles.
  4. Softmax over k on a repartitioned (128,512) tile; val gathered f32
     from DRAM; weighted sums over k via block-diagonal matmuls.
"""
import os
import numpy as np
import ml_dtypes

KSTAGE = int(os.environ.get("KSTAGE", "5"))

B, N, M = 2, 16384, 4096
NCORES = 8
SHARDS_PER_B = NCORES // B          # 4
NS = N // SHARDS_PER_B              # 4096 points per core
C = 64
KNN = 16
EPS = 1e-5
NT = NS // 128                      # 32 point tiles per core
PAIRS = NS * KNN                    # 65536 pairs per core
BLK = 2048                          # pairs per point-tile

bf16 = ml_dtypes.bfloat16


def _pair(x):
    x = np.asarray(x, np.float32)
    h = x.astype(bf16).astype(np.float32)
    l = (x - h).astype(bf16)
    return h.astype(bf16), l


def _triple(x):
    x = np.asarray(x, np.float32)
    h = x.astype(bf16).astype(np.float32)
    m = (x - h).astype(bf16).astype(np.float32)
    l = (x - h - m).astype(bf16)
    return h.astype(bf16), m.astype(bf16), l


_NC_CACHE = {}


def _build_nc():
    if "nc" in _NC_CACHE:
        return _NC_CACHE["nc"]
    import concourse.bacc as bacc
    import concourse.tile as tile
    import concourse.mybir as mybir
    from contextlib import ExitStack

    dt = mybir.dt
    AF = mybir.ActivationFunctionType
    AL = mybir.AluOpType

    nc = bacc.Bacc("TRN2", target_bir_lowering=False)

    def mm(out_ap, lhsT_ap, rhs_ap, start=True, stop=True, skip_group_check=False):
        # chunk moving/free dim to 512 (one PSUM bank per matmul)
        fs = rhs_ap.shape[-1]
        for f0 in range(0, fs, 512):
            f1 = min(f0 + 512, fs)
            nc.tensor.matmul(out_ap[..., f0:f1], lhsT_ap, rhs_ap[..., f0:f1],
                             start=start, stop=stop,
                             skip_group_check=skip_group_check)

    def din(name, shape, dtype):
        return nc.declare_dram_parameter(name, list(shape), dtype, isOutput=False)

    f32, bf, i16, u32 = dt.float32, dt.bfloat16, dt.int16, dt.uint32

    xd = din("xd", (30, NS), bf)
    yd = din("yd", (30, M), bf)
    xyzaP = din("xyzaP", (11, NS), bf)    # [xh(3) xh(3) xl(3) 1 1] for A''
    xlraP = din("xlraP", (9, M), bf)      # [yh(3) yh(3) yl(3)] for C''
    sft_b = din("sft_b", (6, M), bf)
    wAP = din("wAP", (11, C), bf)         # [wh wl wh bias_h bias_l]
    wCP = din("wCP", (9, C), bf)          # [ch cl ch], c = -s*rp_w1^T
    wge1 = din("wge1", (3, C), bf)        # ge_w1^T (lr side, rhs = yd[0:3])
    wge1h = din("wge1h", (3, C), bf)      # 0.5*ge_w1^T (hr side, rhs = xd[0:3])
    ge1s = din("ge1s", (C, 1), f32)
    ge1bb = din("ge1bb", (C, 1), f32)
    wge2 = din("wge2", (C, C), bf)
    ge2b = din("ge2b", (C, 1), f32)
    wsc = din("wsc", (6, C), bf)
    scbb = din("scbb", (C, 1), f32)
    wsh = din("wsh", (6, C), bf)
    shbb = din("shbb", (C, 1), f32)
    wk = din("wk", (C, C), bf)
    kbb = din("kbb", (C, 1), f32)
    wq = din("wq", (C, C), bf)
    qbb = din("qbb", (C, 1), f32)
    wq2 = din("wq2", (C, C), bf)
    wb2 = din("wb2", (C, 1), bf)
    wbd1 = din("wbd1", (C, C), bf)
    bd1s = din("bd1s", (C, 1), f32)
    bd1bb = din("bd1bb", (C, 1), f32)
    wbd2 = din("wbd2", (C, 1), bf)
    bd2b = din("bd2b", (1, 1), f32)
    eye64b = din("eye64b", (128, C), bf)  # bottom half = I64
    eye128b = din("eye128b", (128, 128), bf)
    eye128f = din("eye128f", (128, 128), f32)
    eye128u = din("eye128u", (128, 128), dt.uint16)
    Pexp = din("Pexp", (128, BLK), bf)
    Jt1 = din("Jt1", (C, 63), bf)
    Jt2 = din("Jt2", (128, 126), bf)
    Jv = din("Jv", (128, 248), bf)
    valt = din("valt", (M, C), f32)

    out_pm = nc.declare_dram_parameter("out_pm", [NS, C], f32, isOutput=True)
    bdy_o = nc.declare_dram_parameter("bdy_o", [1, NS], f32, isOutput=True)
    idx_o = nc.declare_dram_parameter("idx_o", [16, PAIRS // 16], dt.int16,
                                      isOutput=True)

    from concourse import library_config
    with tile.TileContext(nc) as tc, ExitStack() as ctx:
        with tc.tile_critical():
            nc.gpsimd.load_library(library_config.mlp)
        cst = ctx.enter_context(tc.tile_pool(name="cst", bufs=1))

        def load(ap, dtype):
            t = cst.tile(list(ap.shape), dtype, tag=ap.name)
            nc.sync.dma_start(t[:], ap[:])
            return t

        s_xd = load(xd, bf)
        s_yd = load(yd, bf)
        s_xyzaP = load(xyzaP, bf)
        s_xlraP = load(xlraP, bf)
        s_sft = load(sft_b, bf)
        s_wAP = load(wAP, bf)
        s_wCP = load(wCP, bf)
        s_wge1 = load(wge1, bf)
        s_wge1h = load(wge1h, bf)
        s_ge1s = load(ge1s, f32)
        s_ge1bb = load(ge1bb, f32)
        s_wge2 = load(wge2, bf)
        s_ge2b = load(ge2b, f32)
        s_wsc = load(wsc, bf)
        s_scbb = load(scbb, f32)
        s_wsh = load(wsh, bf)
        s_shbb = load(shbb, f32)
        s_wk = load(wk, bf)
        s_kbb = load(kbb, f32)
        s_wq = load(wq, bf)
        s_qbb = load(qbb, f32)
        s_wq2 = load(wq2, bf)
        s_wb2 = load(wb2, bf)
        s_wbd1 = load(wbd1, bf)
        s_bd1s = load(bd1s, f32)
        s_bd1bb = load(bd1bb, f32)
        s_wbd2 = load(wbd2, bf)
        s_bd2b = load(bd2b, f32)
        s_e64 = load(eye64b, bf)
        s_e128b = load(eye128b, bf)
        s_e128f = load(eye128f, f32)
        s_e128u = load(eye128u, dt.uint16)
        s_P = load(Pexp, bf)
        s_Jt1 = load(Jt1, bf)
        s_Jt2 = load(Jt2, bf)
        s_Jv = load(Jv, bf)

        per = ctx.enter_context(tc.tile_pool(name="per", bufs=1))
        table = per.tile([128, M // 128, 128], bf, tag="table")
        Qhi = per.tile([C, NS], bf, tag="Qhi")
        Qts = per.tile([128, NS // 2], bf, tag="Qts")
        A1f = per.tile([128, NT, C], bf, tag="A1f")
        A2f = per.tile([128, NT, C], bf, tag="A2f")
        idxl = per.tile([128, PAIRS // 16], i16, tag="idxl")
        if32a = per.tile([128, NT * 16], f32, tag="if32a")
        idxs0 = per.tile([16, PAIRS // 16], i16, tag="idxs0")
        t1s = per.tile([32, BLK], f32, tag="t1s")
        t2s = per.tile([C, BLK // 2], f32, tag="t2s")
        attnT = per.tile([128, PAIRS // 128], f32, tag="attnT")
        qbt = per.tile([128, 32], f32, tag="qbt")

        # ---------- phase 1: conv chains, tables ----------
        with tc.tile_pool(name="ph1", bufs=1) as ph1, \
             tc.tile_pool(name="ps1", bufs=2, space="PSUM") as ps1, \
             tc.tile_pool(name="wk1", bufs=2) as wk1:

            def conv(dst_ap, lhsT, rhs_ap, width, act_func, scale=1.0, bias=0.0,
                     chunk=1024):
                for c0 in range(0, width, chunk):
                    w = min(chunk, width - c0)
                    pt = ps1.tile([C, chunk], f32, tag="psA")
                    mm(pt[:, :w], lhsT, rhs_ap[:, c0:c0 + w])
                    nc.scalar.activation(dst_ap[:, c0:c0 + w], pt[:, :w],
                                         act_func, bias=bias, scale=scale)

            g1 = ph1.tile([C, M], bf, tag="g1")
            glrb = ph1.tile([C, M], bf, tag="glrb")
            glmh = ph1.tile([C, M], bf, tag="glmh")

            # lr chain
            conv(g1[:, :], s_wge1[:], s_yd[0:3, :], M, AF.Relu,
                 scale=s_ge1s[:, 0:1], bias=s_ge1bb[:, 0:1])
            conv(glrb[:, :], s_wge2[:], g1[:], M, AF.Identity, bias=s_ge2b[:, 0:1])
            # glm = glr*(sc+1) + sh, sc/sh straight from PSUM via STT
            for c0 in range(0, M, 1024):
                psc = ps1.tile([C, 1024], f32, tag="psA")
                mm(psc[:], s_wsc[:], s_sft[:, c0:c0 + 1024])
                psh = ps1.tile([C, 1024], f32, tag="psA")
                mm(psh[:], s_wsh[:], s_sft[:, c0:c0 + 1024])
                sl = slice(c0, c0 + 1024)
                nc.vector.scalar_tensor_tensor(
                    glmh[:, sl], psc[:], s_scbb[:, 0:1], glrb[:, sl],
                    op0=AL.add, op1=AL.mult)
                nc.vector.scalar_tensor_tensor(
                    glmh[:, sl], psh[:], s_shbb[:, 0:1], glmh[:, sl],
                    op0=AL.add, op1=AL.add)

            # K | C'' table (transposed, 256B rows)
            for t in range(M // 128):
                sl = slice(t * 128, t * 128 + 128)
                pk = ps1.tile([C, 128], f32, tag="psB")
                mm(pk[:], s_wk[:], glmh[:, sl])
                pc = ps1.tile([C, 128], f32, tag="psB")
                mm(pc[:], s_wCP[:], s_xlraP[:, sl])
                kc = wk1.tile([128, 128], bf, tag="kc")
                nc.scalar.activation(kc[0:C, :], pk[:], AF.Identity,
                                     bias=s_kbb[:, 0:1])
                nc.scalar.copy(kc[C:128, :], pc[:])
                ptr = ps1.tile([128, 128], bf, tag="psT")
                nc.tensor.transpose(ptr[:], kc[:], s_e128b[:])
                nc.scalar.copy(table[:, t, :], ptr[:])

            # hr chain (g1 slot reused for hidden layers)
            g1h = ph1.tile([C, NS], bf, tag="g1")
            gh = ph1.tile([C, NS], bf, tag="gh")
            conv(g1h[:, :], s_wge1h[:], s_xd[0:3, :], NS, AF.Relu,
                 scale=s_ge1s[:, 0:1], bias=s_ge1bb[:, 0:1])
            conv(gh[:, :], s_wge2[:], g1h[:], NS, AF.Identity, bias=s_ge2b[:, 0:1])
            conv(Qhi[:, :], s_wq[:], gh[:], NS, AF.Identity, bias=s_qbb[:, 0:1])
            hbt = ph1.tile([C, NS], bf, tag="g1")
            conv(hbt[:, :], s_wbd1[:], gh[:], NS, AF.Relu, scale=s_bd1s[:, 0:1],
                 bias=s_bd1bb[:, 0:1])
            for c0 in range(0, NS, 1024):
                pb = ps1.tile([C, 1024], f32, tag="psA")
                mm(pb[0:1, :], s_wbd2[:], hbt[:, c0:c0 + 1024])
                bsb = wk1.tile([1, 1024], f32, tag="bsb")
                nc.scalar.activation(bsb[:], pb[0:1, :], AF.Sigmoid,
                                     bias=s_bd2b[0:1, 0:1])
                nc.sync.dma_start(bdy_o[0:1, c0:c0 + 1024], bsb[:])
                pq2 = ps1.tile([C, 1024], f32, tag="psA")
                mm(pq2[0:1, :], s_wb2[:], Qhi[:, c0:c0 + 1024])
                qst = wk1.tile([1, 1024], f32, tag="qst")
                nc.scalar.copy(qst[:], pq2[0:1, :])
                cb = c0 // 32
                nc.sync.dma_start(qbt[cb:cb + 32, :], qst[:])
                pqt = ps1.tile([C, 1024], f32, tag="psA")
                mm(pqt[:], s_wq2[:], Qhi[:, c0:c0 + 1024])
                qtv = pqt[:].rearrange("c (t h j) -> c t h j", h=2, j=C)
                tsl = slice(c0 // 128, c0 // 128 + 8)
                d0 = Qts[0:C, :].rearrange("c (t j) -> c t j", j=C)
                d1 = Qts[C:128, :].rearrange("c (t j) -> c t j", j=C)
                nc.scalar.copy(d0[:, tsl, :], qtv[:, :, 0, :])
                nc.scalar.copy(d1[:, tsl, :], qtv[:, :, 1, :])

            # A'' tiles (point-major) + bf16 pair split
            for t in range(NT):
                pa = ps1.tile([128, C], f32, tag="psT")
                mm(pa[:], s_xyzaP[:, t * 128:(t + 1) * 128], s_wAP[:])
                nc.scalar.copy(A1f[:, t, :], pa[:])
                nc.vector.tensor_tensor(A2f[:, t, :], pa[:], A1f[:, t, :],
                                        op=AL.subtract)

        # ---------- phase 2: distances + top-16 ----------
        if KSTAGE < 2:
            nc.gpsimd.memset(if32a[:, :], 0.0)
        with tc.tile_pool(name="psS", bufs=1, space="PSUM") as psS, \
             tc.tile_pool(name="wkS", bufs=4) as wkS:
            for t in range(NT if KSTAGE >= 2 else 0):
                ps = psS.tile([128, M], f32, tag="s")
                mm(ps[:], s_xd[:, t * 128:(t + 1) * 128], s_yd[:])
                v1 = wkS.tile([128, 8], f32, tag="v1")
                i1 = wkS.tile([128, 8], u32, tag="i1")
                v2 = wkS.tile([128, 8], f32, tag="v2")
                i2 = wkS.tile([128, 8], u32, tag="i2")
                nc.vector.max(v1[:], ps[:])
                nc.vector.max_index(i1[:], v1[:], ps[:])
                nc.vector.match_replace(ps[:], v1[:], ps[:], -3.0e38)
                nc.vector.max(v2[:], ps[:])
                nc.vector.max_index(i2[:], v2[:], ps[:])
                nc.vector.tensor_copy(if32a[:, t * 16:t * 16 + 8], i1[:])
                nc.vector.tensor_copy(if32a[:, t * 16 + 8:t * 16 + 16], i2[:])

        # ---------- phase 3: gather + attention ----------
        with tc.tile_pool(name="psB", bufs=1, space="PSUM") as psB, \
             tc.tile_pool(name="psU", bufs=1, space="PSUM") as psU, \
             tc.tile_pool(name="psV", bufs=1, space="PSUM") as psV, \
             tc.tile_pool(name="wkB", bufs=2) as wkB:
            ptt = psB.tile([128, BLK], f32, tag="ptt")
            pt1 = ptt[0:32, :]
            pt2 = ptt[64:128, 0:BLK // 2]
            for t in range(NT if KSTAGE >= 3 else 0):
                pit = psV.tile([16, 128], f32, tag="psV")
                nc.tensor.transpose(pit[:], if32a[:, t * 16:(t + 1) * 16],
                                    s_e128f[:])
                nc.vector.tensor_copy(idxs0[:, t * 128:(t + 1) * 128], pit[:])
            if KSTAGE >= 3:
                nc.sync.dma_start(idx_o[:, :], idxs0[:])
                for gi in range(8):
                    nc.sync.dma_start(idxl[16 * gi:16 * gi + 16, :], idxs0[:])
            else:
                nc.gpsimd.memset(idxl[:, :], 0)
            for t in range(NT if KSTAGE >= 4 else 0):
                isl = idxl[:, t * 128:(t + 1) * 128]
                g = wkB.tile([128, 1, BLK], bf, tag="g")
                nc.gpsimd.dma_gather(
                    g[:], table[:].rearrange("p t c -> p (t c)"), isl,
                    num_idxs=BLK, num_idxs_reg=BLK, elem_size=128,
                    transpose=True, sbuf_tokens_per_rank=128,
                    sbuf_free_dim_per_rank=256)
                p1 = wkB.tile([C, BLK], bf, tag="p1")
                qv = Qhi[:, t * 128:(t + 1) * 128].unsqueeze(2).broadcast_to(
                    (C, 128, KNN))
                nc.vector.tensor_mul(
                    p1[:].rearrange("c (n k) -> c n k", k=KNN),
                    g[0:C, 0, :].rearrange("c (n k) -> c n k", k=KNN), qv)
                mm(pt1, s_Jt1[:, 31 - t:63 - t], p1[:],
                   start=(t == 0), stop=(t == NT - 1), skip_group_check=True)
                hpk = wkB.tile([128, BLK // 2], bf, tag="hpk")
                for h in range(2):
                    hs = slice(h * (BLK // 2), (h + 1) * (BLK // 2))
                    pu = psU.tile([C, BLK // 2], f32, tag="psU")
                    mm(pu[:], s_e64[C:128, :], g[C:128, 0, hs],
                       start=True, stop=False)
                    mm(pu[:], A1f[:, t, :], s_P[:, hs], start=False, stop=False)
                    mm(pu[:], A2f[:, t, :], s_P[:, hs], start=False, stop=True)
                    nc.scalar.activation(hpk[h * C:(h + 1) * C, :], pu[:], AF.Relu)
                p2 = wkB.tile([128, BLK // 2], bf, tag="p2")
                qtv = Qts[:, t * C:(t + 1) * C].unsqueeze(2).broadcast_to(
                    (128, C, KNN))
                nc.vector.tensor_mul(
                    p2[:].rearrange("c (n k) -> c n k", k=KNN),
                    hpk[:].rearrange("c (n k) -> c n k", k=KNN), qtv)
                mm(pt2, s_Jt2[:, 62 - 2 * t:126 - 2 * t], p2[:],
                   start=(t == 0), stop=(t == NT - 1), skip_group_check=True)
            if KSTAGE >= 4:
                nc.scalar.copy(t1s[:], pt1)
                nc.scalar.copy(t2s[:], pt2)
            else:
                nc.gpsimd.memset(t1s[:], 0.0)
                nc.gpsimd.memset(t2s[:], 0.0)

            # softmax over k on repartitioned logits
            if KSTAGE < 5:
                nc.gpsimd.memset(attnT[:, :], 0.0)
            lg = wkB.tile([128, PAIRS // 128], f32, tag="lg")
            if KSTAGE < 5:
                nc.gpsimd.memset(lg[:, :], 0.0)
            if KSTAGE >= 5:
                nc.sync.dma_start(lg[:], t1s[:])
            if KSTAGE >= 5:
                lg2 = wkB.tile([128, PAIRS // 128], f32, tag="lg2")
                nc.sync.dma_start(lg2[:], t2s[:])
                nc.vector.tensor_add(lg[:], lg[:], lg2[:])
                lgv = lg[:].rearrange("p (a k) -> p a k", k=KNN)
                nc.vector.tensor_add(lgv, lgv,
                                     qbt[:].unsqueeze(2).broadcast_to((128, 32, KNN)))
                ex = wkB.tile([128, PAIRS // 128], f32, tag="ex")
                nc.scalar.activation(ex[:], lg[:], AF.Exp, scale=0.125)
                den = wkB.tile([128, 32], f32, tag="den")
                nc.vector.reduce_sum(den[:],
                                     ex[:].rearrange("p (a k) -> p a k", k=KNN),
                                     axis=mybir.AxisListType.X)
                rden = wkB.tile([128, 32], f32, tag="rden")
                nc.vector.reciprocal(rden[:], den[:])
                at = wkB.tile([128, PAIRS // 128], f32, tag="at")
                nc.vector.tensor_mul(at[:].rearrange("p (a k) -> p a k", k=KNN),
                                     ex[:].rearrange("p (a k) -> p a k", k=KNN),
                                     rden[:].unsqueeze(2).broadcast_to((128, 32, KNN)))
            for r in range(4 if KSTAGE >= 5 else 0):
                pat = psV.tile([128, 128], f32, tag="psV")
                nc.tensor.transpose(pat[:], at[:, r * 128:(r + 1) * 128],
                                    s_e128f[:])
                nc.scalar.copy(
                    attnT[:].rearrange("c (p r) -> c p r", r=4)[:, :, r], pat[:])

            # val gather + weighted k-sum
            for t in range(NT if KSTAGE >= 5 else 0):
                isl = idxl[:, t * 128:(t + 1) * 128]
                vg = wkB.tile([128, KNN, C], f32, tag="vg")
                nc.gpsimd.dma_gather(vg[:], valt[:, :], isl, num_idxs=BLK,
                                     num_idxs_reg=BLK, elem_size=C,
                                     transpose=False)
                po = psV.tile([128, C], f32, tag="psV")
                for j in range(KNN):
                    sv = wkB.tile([128, C], bf, tag="sv")
                    nc.vector.tensor_scalar_mul(
                        sv[:], vg[:, j, :], attnT[:, t * KNN + j:t * KNN + j + 1])
                    nc.tensor.matmul(po[:], s_Jv[:, 120 - 8 * j:248 - 8 * j],
                                     sv[:], start=(j == 0), stop=(j == KNN - 1))
                osb = wkB.tile([128, C], f32, tag="osb")
                nc.scalar.copy(osb[:], po[:])
                nc.sync.dma_start(out_pm[t * 128:(t + 1) * 128, :], osb[:])

    nc.compile()
    _NC_CACHE["nc"] = nc
    return nc


def _host_prep(inputs):
    i = {k: np.asarray(v, np.float32) for k, v in inputs.items()}
    s_bn = (i['rp_g'] / np.sqrt(i['rp_v'] + EPS)).astype(np.float32)
    t_bn = (i['rp_be'] - i['rp_m'] * s_bn).astype(np.float32)
    ge1s = (i['ge_g1'] / np.sqrt(i['ge_v1'] + EPS)).astype(np.float32)
    ge1bb = (i['ge_be1'] - i['ge_m1'] * ge1s + ge1s * i['ge_b1']).astype(np.float32)
    bd1s = (i['bd_g'] / np.sqrt(i['bd_v'] + EPS)).astype(np.float32)
    bd1bb = (i['bd_be'] - i['bd_m'] * bd1s + bd1s * i['bd_b1']).astype(np.float32)

    wa = (i['rp_w1'] * s_bn[:, None]).T.astype(np.float32)       # (3, 64)
    wah, wal = _pair(wa)
    abias = (s_bn * i['rp_b1'] + t_bn).astype(np.float32)
    abh, abl = _pair(abias)
    wc = (-i['rp_w1'] * s_bn[:, None]).T.astype(np.float32)
    wch, wcl = _pair(wc)

    eye64b = np.concatenate([np.zeros((C, C)), np.eye(C)], 0).astype(bf16)
    eye128b = np.eye(128, dtype=bf16)
    eye128f = np.eye(128, dtype=np.float32)
    Pexp = np.zeros((128, BLK), np.float32)
    for p in range(128):
        Pexp[p, p * KNN:(p + 1) * KNN] = 1.0
    Jt1 = np.zeros((C, 63), np.float32)
    Jt1[:, 31] = 1.0
    Jt2 = np.zeros((128, 126), np.float32)
    Jt2[0:C, 62] = 1.0
    Jt2[C:128, 63] = 1.0
    Jv = np.zeros((128, 248), np.float32)
    for r in range(8):
        Jv[r * 16:(r + 1) * 16, 120 + r] = 1.0

    shared = dict(
        wAP=np.concatenate([wah, wal, wah, abh[None, :], abl[None, :]], 0),
        wCP=np.concatenate([wch, wcl, wch], 0),
        wge1=i['ge_w1'].T.astype(bf16),
        wge1h=(0.5 * i['ge_w1'].T).astype(bf16),
        ge1s=ge1s[:, None], ge1bb=ge1bb[:, None],
        wge2=i['ge_w2'].T.astype(bf16), ge2b=i['ge_b2'][:, None],
        wsc=i['sc_w'].T.astype(bf16), scbb=(i['sc_b'] + 1.0)[:, None],
        wsh=i['sh_w'].T.astype(bf16), shbb=i['sh_b'][:, None],
        wk=i['k_w'].T.astype(bf16), kbb=i['k_b'][:, None],
        wq=i['q_w'].T.astype(bf16), qbb=i['q_b'][:, None],
        wq2=i['rp_w2'].astype(bf16),
        wb2=i['rp_b2'][:, None].astype(bf16),
        wbd1=i['bd_w1'].T.astype(bf16),
        bd1s=bd1s[:, None], bd1bb=bd1bb[:, None],
        wbd2=i['bd_w2'].T.astype(bf16), bd2b=i['bd_b2'][None, :],
        eye64b=eye64b, eye128b=eye128b, eye128f=eye128f,
        eye128u=np.eye(128, dtype=np.uint16),
        Pexp=Pexp.astype(bf16), Jt1=Jt1.astype(bf16), Jt2=Jt2.astype(bf16),
        Jv=Jv.astype(bf16),
    )

    in_maps = []
    for c in range(NCORES):
        b = c // SHARDS_PER_B
        sl = slice((c % SHARDS_PER_B) * NS, (c % SHARDS_PER_B + 1) * NS)
        xhr = i['xyz_hr'][b][:, sl]
        xlr = i['xyz_lr'][b]
        xh, xm, xl = _triple(2.0 * xhr)
        yh, ym, yl = _triple(xlr)
        nrm = (xlr ** 2).sum(0).astype(np.float32)
        nh, nm, nl = _triple(nrm)
        negs = np.full((3, NS), -1.0, bf16)
        # row pairing: [(xh,yh)x3 | (-1, n_hml) | remaining (a,b) pairings]
        xrows = [xh, negs]
        yrows = [yh, np.stack([nh, nm, nl], 0)]
        for a, bb in [(xh, ym), (xh, yl), (xm, yh), (xm, ym), (xm, yl),
                      (xl, yh), (xl, ym), (xl, yl)]:
            xrows.append(np.asarray(a, bf16))
            yrows.append(np.asarray(bb, bf16))
        xd_ = np.concatenate(xrows, 0)
        yd_ = np.concatenate(yrows, 0)
        xah, xal = _pair(xhr)
        yah, yal = _pair(xlr)
        m = dict(shared)
        m.update(
            xd=np.ascontiguousarray(xd_), yd=np.ascontiguousarray(yd_),
            xyzaP=np.concatenate([xah, xah, xal, np.ones((2, NS), bf16)], 0),
            xlraP=np.concatenate([yah, yah, yal], 0),
            sft_b=i['sft_feat_lr'][b].astype(bf16),
            valt=np.ascontiguousarray(i['val_lr'][b].T),
        )
        in_maps.append(m)
    return in_maps


def _host_attention(i, b, sl, k_idx):
    """f32 numpy attention tail for one shard (host fallback when the
    on-device gather path is unavailable)."""
    x = i['xyz_hr'][b][:, sl]
    y = i['xyz_lr'][b]

    def conv(w, bb, z):
        return w @ z + bb[:, None]

    def bn(g, be, m, v, z):
        s = g / np.sqrt(v + EPS)
        return s[:, None] * z + (be - m * s)[:, None]

    def geom(z):
        h = np.maximum(bn(i['ge_g1'], i['ge_be1'], i['ge_m1'], i['ge_v1'],
                          conv(i['ge_w1'], i['ge_b1'], z)), 0)
        return conv(i['ge_w2'], i['ge_b2'], h)

    gx = geom(x)
    gy = geom(y)
    sc = conv(i['sc_w'], i['sc_b'], i['sft_feat_lr'][b])
    sh = conv(i['sh_w'], i['sh_b'], i['sft_feat_lr'][b])
    glm = gy * (sc + 1.0) + sh
    Q = conv(i['q_w'], i['q_b'], gx)
    Kf = conv(i['k_w'], i['k_b'], glm)
    s_bn = i['rp_g'] / np.sqrt(i['rp_v'] + EPS)
    t_bn = i['rp_be'] - i['rp_m'] * s_bn
    A = s_bn[:, None] * (i['rp_w1'] @ x + i['rp_b1'][:, None]) + t_bn[:, None]
    Cm = s_bn[:, None] * (i['rp_w1'] @ y)
    Qt = i['rp_w2'].T @ Q
    qb2 = i['rp_b2'] @ Q
    Kg = Kf[:, k_idx]
    h = np.maximum(A[:, :, None] - Cm[:, k_idx], 0)
    t1 = np.einsum('cn,cnk->nk', Q, Kg)
    t2 = np.einsum('cn,cnk->nk', Qt, h)
    logits = (t1 + t2 + qb2[:, None]) * 0.125
    e = np.exp(logits - logits.max(-1, keepdims=True))
    attn = e / e.sum(-1, keepdims=True)
    Vg = i['val_lr'][b][:, k_idx]
    return np.einsum('nk,cnk->cn', attn, Vg).astype(np.float32)


def kernel(**inputs):
    from concourse.bass_utils import run_bass_kernel_spmd
    nc = _build_nc()
    in_maps = _host_prep(inputs)
    res = run_bass_kernel_spmd(nc, in_maps, list(range(NCORES)))
    out = np.zeros((B, C, N), np.float32)
    bdy = np.zeros((B, 1, N), np.float32)
    i = {k: np.asarray(v, np.float32) for k, v in inputs.items()}
    for c in range(NCORES):
        b = c // SHARDS_PER_B
        sl = slice((c % SHARDS_PER_B) * NS, (c % SHARDS_PER_B + 1) * NS)
        bdy[b, 0, sl] = res.results[c]["bdy_o"][0]
        if KSTAGE >= 5:
            out[b, :, sl] = res.results[c]["out_pm"].T
        else:
            flat = res.results[c]["idx_o"].T.reshape(-1)      # (s p) unwrap
            k_idx = flat.astype(np.int64).reshape(NS, KNN)
            out[b, :, sl] = _host_attention(i, b, sl, k_idx)
    return out, bdy
